# revision 15
# baseline (speedup 1.0000x reference)
"""MoE block (top-2 routed 3x3 conv experts) Trainium2 Bass kernel.

Strategy: data-parallel over batch, 2 samples per core on 8 cores.
The conv is linear in the kernel, so the top-2 expert kernels are
combined with the routing probabilities first (w_comb = sum_e p_e W_e),
then one 3x3 SAME conv per sample runs as matmuls.

v4 design:
- fp16 conv path (x, expert weights, output staging); fp32 gate.
- x padded to [C, 130*130] on the HOST so every input DMA is one fully
  contiguous flat range per partition (per-row 256B segments sit below
  the 512B SDMA line-rate threshold and run at ~50 GB/s).
- Partitions 0-63 = padded channels; 64-127 = same data shifted +2
  elements, so a [128, 4x128] rhs read gives taps (dy,-1) on top and
  (dy,+1) on the bottom half. dx=0 taps are separate K=64 matmuls on
  the top half only.
- Residual folded into the center-tap expert weights on host
  (W_e[center] += I; routing probs sum to 1), so the post-conv op is a
  pure bias add: ACT Identity+bias / DVE tensor_scalar_add alternating.
- Column-tiled conv in 2-pair blocks: 4 tiles (16 output rows) per
  block, tiles alternating between PSUM partition halves / PE column
  groups, SLOT-MAJOR so 4 consecutive matmuls share one weight load
  (walrus --enable-ldw-opt=true dedups the LDWEIGHTS; per-MM weight
  loads otherwise serialize each column group's chain).
- Pooled partials in eight ~2.1K-element pieces spread over ACT, DVE
  and GpSimd so they pipeline with the input DMAs; the combined-weight
  chains run split: pair taps [128,3C] on DVE, single taps [64,3C] on
  GpSimd (GpSimd has no PSUM port, so w8 is staged through SBUF).
"""
import numpy as np
from contextlib import ExitStack

import concourse.bass as bass
import concourse.bass_utils as _bu
import concourse.tile as tile
from concourse import bacc, mybir
from concourse.bass_utils import run_bass_kernel_spmd
from concourse.tile import add_dep_helper

F32 = mybir.dt.float32
F16 = mybir.dt.float16
AX = mybir.AxisListType
OP = mybir.AluOpType
ACTF = mybir.ActivationFunctionType

B, C, H, W, E, GH = 16, 64, 128, 128, 8, 16
NCORES = 8
SPB = B // NCORES          # samples per core
HP, WP = H + 2, W + 2      # 130
FLAT = HP * WP             # 16900
NB = H // 16               # 8 conv blocks (2 pairs = 16 output rows each)
QS = FLAT // 4             # 4225: flat quarter-chunk
HQ = QS // 2               # 2112: pooled piece size
BEND = FLAT - 2            # last valid element of the shifted bottom copy

_cache = {}


def _emit_sample_loads(nc, pools, s, XX, xs_ap, mid_sp=None):
    """Input DMAs + pooled partial sums for sample s.

    Top copy (partitions 0:64) = xpad quarters on SP; bottom copy
    (partitions 64:128, shifted +2) on ACT, quarters 2,3 first. Pooled
    partials in 8 pieces: ACT takes xpad[0:3HQ), GpSimd [3HQ:4HQ) via
    the top copy; DVE takes xpad[2QS:2QS+3HQ), GpSimd the rest via the
    bottom copy (bottom dst [a:b) holds xpad[a+2:b+2)).
    """
    f = pools

    def top(k):
        nc.sync.dma_start(
            XX[0:64, QS * k : QS * (k + 1)],
            xs_ap[s, :, QS * k : QS * (k + 1)],
        )

    def bot(k):
        a = max(QS * k - 2, 0)
        b = min(QS * (k + 1) - 2, BEND)
        return nc.scalar.dma_start(XX[64:128, a:b], xs_ap[s, :, a + 2 : b + 2])

    part = f["gate"].tile([128, 2], F32, tag="part", name=f"part{s}")
    nc.vector.memset(part[:], 0.0)

    top(0)
    top(1)
    if mid_sp is not None:
        mid_sp()
    top(2)
    top(3)
    bot(2)
    bot(3)
    bot_dmas = [bot(0), bot(1)]

    # pooled partials: ACT Copy+fp32-accum on the top copy; DVE fp16-out
    # [64, 2, N] -> [64, 2] tensor_reduce on the bottom copy + one top
    # piece (fp16 in+out with >1 output element is eligible for the DVE
    # 2x perf mode; internal accumulation is fp32). part16 slots:
    # col 0,1 (bottom) = xpad quarters 2,3; col 2,3 (top) = [6338:8450).
    part16 = f["gate"].tile([128, 4], F16, tag="part16", name=f"part16_{s}")
    nc.vector.memset(part16[:], 0.0)

    def act_piece(a, b, col):
        scr = f["scratch"].tile([64, 2 * HQ + 8], F16, tag="scrA", name=f"scrA{s}_{col}")
        nc.scalar.activation(
            scr[:, 0 : b - a],
            XX[0:64, a:b],
            ACTF.Copy,
            accum_out=part[0:64, col : col + 1],
        )

    def dve_piece(row, a, b, col):
        half = (b - a) // 2
        assert 2 * half == b - a
        view = XX[row : row + 64, a:b].rearrange("p (k n) -> p k n", k=2)
        with nc.allow_low_precision("fp16 pooled partial; accum is fp32 internal"):
            nc.vector.tensor_reduce(
                part16[row : row + 64, col : col + 2], view, axis=AX.X, op=OP.add
            )

    act_piece(0, 3170, 0)                      # xpad[0:3170)
    act_piece(3170, 6338, 1)                   # xpad[3170:6338)
    dve_piece(0, 6338, 2 * QS, 2)              # xpad[6338:8450)  via top
    dve_piece(64, 2 * QS - 2, 12672, 0)        # xpad[8450:12674) via bottom
    dve_piece(64, 12672, BEND, 2)              # xpad[12674:FLAT) via bottom

    # fold: the stacked-wg1 gate matmul adds the partition halves
    pooled = f["gate"].tile([128, 1], F32, tag="pooled", name=f"pooled{s}")
    t16 = f["gate"].tile([128, 2], F32, tag="t16", name=f"t16_{s}")
    t_a = f["gate"].tile([128, 1], F32, tag="t_a", name=f"t_a{s}")
    nc.vector.tensor_tensor(t16[:], part16[:, 0:2], part16[:, 2:4], op=OP.add)
    nc.vector.tensor_tensor(t_a[:], part[:, 0:1], part[:, 1:2], op=OP.add)
    nc.vector.scalar_tensor_tensor(
        pooled[:], t16[:, 0:1], t_a[:], t16[:, 1:2],
        op0=OP.add, op1=OP.add,
    )
    return pooled, bot_dmas


def _emit_sample_gate(nc, pools, s, pooled, consts, h_ext):
    """Gate MLP + softmax + top-2 + combined weights/bias for one sample.

    Uses exp-without-max-sub (logits are small) and folds the top-2 mask
    and renormalization:  w8 = (u>=m2)*u / (sum((u>=m2)*u) + sum(u)*1e-8)
    which equals the reference's normalized-probs formula exactly.
    Returns (wcombrP, wcombrS, b_comb, exp_inst).
    """
    f = pools
    g = f["gate"]
    wg1x2_sb, bg1_sb, wg2_sb, bexp_sb, wpsP_sb, wpsS_sb, ones = consts
    n = lambda base: f"{base}{s}"

    h_ps = f["gpsum"].tile([GH, 1], F32, tag="gps", name=n("h_ps"))
    nc.tensor.matmul(h_ps[:], lhsT=wg1x2_sb[:], rhs=pooled[:], start=True, stop=True)
    nc.vector.tensor_scalar(
        h_ext[0:GH, :], h_ps[:], bg1_sb[:], 0.0, OP.add, OP.max
    )

    lg_ps = f["gpsum"].tile([1, E], F32, tag="gps", name=n("lg_ps"))
    nc.tensor.matmul(lg_ps[:], lhsT=h_ext[:], rhs=wg2_sb[:], start=True, stop=True)

    # u = exp(logits) (unnormalized softmax; |logits| is tiny, no max-sub)
    u = g.tile([1, E], F32, tag="u", name=n("u"))
    exp_inst = nc.scalar.activation(u[:], lg_ps[:], ACTF.Exp)
    usum = g.tile([1, 1], F32, tag="usum", name=n("usum"))
    nc.vector.tensor_reduce(usum[:], u[:], axis=AX.X, op=OP.add)
    # top-2: pm = (u < max)*u (valid since u>0), m2 = 2nd max, spv = (u>=m2)*u
    m1p = g.tile([1, 1], F32, tag="m1p", name=n("m1p"))
    nc.vector.tensor_reduce(m1p[:], u[:], axis=AX.X, op=OP.max)
    pm = g.tile([1, E], F32, tag="pm", name=n("pm"))
    nc.vector.scalar_tensor_tensor(pm[:], u[:], m1p[:], u[:], op0=OP.is_lt, op1=OP.mult)
    m2 = g.tile([1, 1], F32, tag="m2", name=n("m2"))
    nc.vector.tensor_reduce(m2[:], pm[:], axis=AX.X, op=OP.max)
    spv = g.tile([1, E], F32, tag="spv", name=n("spv"))
    nc.vector.scalar_tensor_tensor(spv[:], u[:], m2[:], u[:], op0=OP.is_ge, op1=OP.mult)
    dsum = g.tile([1, 1], F32, tag="dsum", name=n("dsum"))
    nc.vector.tensor_reduce(dsum[:], spv[:], axis=AX.X, op=OP.add)
    dd = g.tile([1, 1], F32, tag="dd", name=n("dd"))
    nc.vector.scalar_tensor_tensor(dd[:], usum[:], 1e-8, dsum[:], op0=OP.mult, op1=OP.add)
    rr = g.tile([1, 1], F32, tag="rr", name=n("rr"))
    nc.vector.reciprocal(rr[:], dd[:])
    w8 = g.tile([1, E], F32, tag="w8", name=n("w8"))
    nc.vector.tensor_scalar_mul(w8[:], spv[:], rr[:])

    # broadcast w8 down all 128 partitions, staged to SBUF (GpSimd has
    # no PSUM port): [128, E] = ones[1,128]^T @ w8[1,E]
    wb_ps = f["gpsum"].tile([128, E], F32, tag="wbps", name=n("wb_ps"), bufs=1)
    nc.tensor.matmul(wb_ps[:], lhsT=ones[:], rhs=w8[:], start=True, stop=True)
    wb_sb = g.tile([128, E], F32, tag="wb_sb", name=n("wb_sb"))
    nc.vector.tensor_copy(wb_sb[:], wb_ps[:])

    # combined bias path (off critical path): b_comb128 = (b_exp
    # duplicated over both partition halves)^T @ w8^T
    w8c_ps = f["gpsum"].tile([E, 1], F32, tag="gps", name=n("w8c_ps"))
    nc.tensor.matmul(w8c_ps[:], lhsT=w8[:], rhs=ones[:, 0:1], start=True, stop=True)
    w8col = g.tile([E, 1], F32, tag="w8col", name=n("w8col"))
    nc.vector.tensor_copy(w8col[:], w8c_ps[:])
    bc_ps = f["gpsum"].tile([128, 1], F32, tag="gps", name=n("bc_ps"))
    nc.tensor.matmul(bc_ps[:], lhsT=bexp_sb[:], rhs=w8col[:], start=True, stop=True)
    b_comb = g.tile([128, 1], F32, tag="b_comb", name=n("b_comb"))
    nc.vector.tensor_copy(b_comb[:], bc_ps[:])

    # combined conv weights, two parallel MAC chains:
    # pair taps [128, 3, C] on DVE; single dx=0 taps [64, 3, C] on
    # GpSimd (slot 1 carries +I for the residual shortcut)
    wcombP = f["wcomb"].tile([128, 3, C], F32, tag="wcombP", name=n("wcombP"))
    nc.vector.tensor_scalar_mul(wcombP[:], wpsP_sb[:, 0], wb_sb[:, 0:1])
    for e in range(1, E):
        nc.vector.scalar_tensor_tensor(
            wcombP[:], wpsP_sb[:, e], wb_sb[:, e : e + 1], wcombP[:],
            op0=OP.mult, op1=OP.add,
        )
    wcombrP = f["wcomb"].tile([128, 3, C], F16, tag="wcombrP", name=n("wcombrP"))
    nc.vector.tensor_copy(wcombrP[:], wcombP[:])

    wcombS = f["wcomb"].tile([64, 3, C], F32, tag="wcombS", name=n("wcombS"))
    nc.vector.tensor_scalar_mul(wcombS[:], wpsS_sb[:, 0], wb_sb[0:64, 0:1])
    for e in range(1, E):
        nc.vector.scalar_tensor_tensor(
            wcombS[:], wpsS_sb[:, e], wb_sb[0:64, e : e + 1], wcombS[:],
            op0=OP.mult, op1=OP.add,
        )
    wcombrS = f["wcomb"].tile([64, 3, C], F16, tag="wcombrS", name=n("wcombrS"))
    nc.vector.tensor_copy(wcombrS[:], wcombS[:])
    return wcombrP, wcombrS, b_comb, exp_inst


def _emit_conv_blocks(nc, pools, s, XX, wcombrP, wcombrS, b_comb, out_ap, blk_range):
    """Conv blocks (4 tiles x 4 output rows) for sample s, slot-major.

    Tiles alternate PSUM partition halves / PE column groups; the four
    matmuls of one weight slot run back-to-back sharing one LDWEIGHTS.
    Posts: ACT (Identity + bias) for bank 0, DVE (tensor_scalar_add)
    for bank 1, into one [128, 8, W] stage tile; two batched out DMAs
    per block cover interleaved 4-row blocks.
    """
    f = pools
    XX3 = XX[:, 0:FLAT].rearrange("p (r c) -> p r c", c=WP)
    # out rows as [blk, bank(2), half(2), row(4)]
    outv = out_ap[s].rearrange("c (blk i j r) w -> c blk i j r w", i=2, j=2, r=4)
    for blk in blk_range:
        banks = [
            f["cpsum"].tile([128, 4, W], F32, tag="cps", name=f"cps{s}_{blk}_{i}")
            for i in (0, 1)
        ]
        stage = f["stage"].tile([128, 2, 4, W], F16, tag="stage", name=f"ost{s}_{blk}")
        for i, ps in enumerate(banks):
            r00 = 16 * blk + 8 * i
            for dyi in range(3):
                for half, r0 in ((0, r00), (64, r00 + 4)):
                    nc.tensor.matmul(
                        ps[half : half + 64],
                        lhsT=wcombrP[:, dyi, :],
                        rhs=XX3[:, r0 + dyi : r0 + dyi + 4, 0:128],
                        start=(dyi == 0),
                        stop=False,
                    )
            for dyi in range(3):
                for half, r0 in ((0, r00), (64, r00 + 4)):
                    nc.tensor.matmul(
                        ps[half : half + 64],
                        lhsT=wcombrS[:, dyi, :],
                        rhs=XX3[0:64, r0 + dyi : r0 + dyi + 4, 1:129],
                        start=False,
                        stop=(dyi == 2),
                    )
            if i == 0:
                nc.scalar.activation(
                    stage[:, 0], ps[:], ACTF.Identity, bias=b_comb[:, 0:1]
                )
            else:
                nc.vector.tensor_scalar_add(stage[:, 1], ps[:], b_comb[:, 0:1])
        # half j=0: banks' partitions 0:64 -> row-blocks (blk, 0/1, 0)
        nc.sync.dma_start(outv[:, blk, :, 0], stage[0:64])
        nc.scalar.dma_start(outv[:, blk, :, 1], stage[64:128])


def build_program():
    if "nc" in _cache:
        return _cache["nc"]
    nc = bacc.Bacc("TRN2", target_bir_lowering=False, debug=False, enable_asserts=False)
    xs_ap = nc.dram_tensor("xs", [SPB, C, FLAT], F16, kind="ExternalInput").ap()
    wpsP_d = nc.dram_tensor("wpsP", [128, E, 3, C], F16, kind="ExternalInput").ap()
    wpsS_d = nc.dram_tensor("wpsS", [64, E, 3, C], F16, kind="ExternalInput").ap()
    wg1_d = nc.dram_tensor("wg1", [128, GH], F32, kind="ExternalInput").ap()
    bg1_d = nc.dram_tensor("bg1", [GH, 1], F32, kind="ExternalInput").ap()
    wg2_d = nc.dram_tensor("wg2", [GH + 1, E], F32, kind="ExternalInput").ap()
    bexp_d = nc.dram_tensor("b_exp", [E, 128], F32, kind="ExternalInput").ap()
    out_ap = nc.dram_tensor("out", [SPB, C, H, W], F16, kind="ExternalOutput").ap()

    with tile.TileContext(nc) as tc, ExitStack() as ctx:
        pools = {
            "const": ctx.enter_context(tc.tile_pool(name="const", bufs=1)),
            "xx": ctx.enter_context(tc.tile_pool(name="xx", bufs=SPB)),
            "gate": ctx.enter_context(tc.tile_pool(name="gate", bufs=2)),
            "wcomb": ctx.enter_context(tc.tile_pool(name="wcomb", bufs=2)),
            "stage": ctx.enter_context(tc.tile_pool(name="stage", bufs=4)),
            "scratch": ctx.enter_context(tc.tile_pool(name="scratch", bufs=1)),
            "gpsum": ctx.enter_context(tc.tile_pool(name="gpsum", bufs=1, space="PSUM")),
            "cpsum": ctx.enter_context(tc.tile_pool(name="cpsum", bufs=6, space="PSUM")),
        }
        cp = pools["const"]
        XX0 = pools["xx"].tile([128, FLAT], F16, tag="XX", name="XX0")
        XX1 = pools["xx"].tile([128, FLAT], F16, tag="XX", name="XX1")
        ones = cp.tile([1, 128], F32)
        nc.gpsimd.memset(ones[:], 1.0)
        # prewarm the ACT exp table before the ACT lane fills with DMAs
        warm = cp.tile([1, 1], F32)
        nc.scalar.activation(warm[:], ones[:, 0:1], ACTF.Exp)
        # h_ext = [relu(...); 1.0] buffers: write the trailing 1.0 rows
        # once, off the gate critical path (SWDGE fixed cost ~1-2us)
        g = pools["gate"]
        h_exts = [g.tile([GH + 1, 1], F32, tag="h_ext", name=f"h_ext{s}") for s in (0, 1)]
        nc.gpsimd.dma_start(h_exts[0][GH : GH + 1, 0:1], ones[0:1, 0:1])
        nc.gpsimd.dma_start(h_exts[1][GH : GH + 1, 0:1], ones[0:1, 0:1])
        # tiny gate weights + single-tap expert weights on the SWDGE lane
        wg1x2_sb = cp.tile([128, GH], F32)
        nc.gpsimd.dma_start(wg1x2_sb[:], wg1_d[:])
        bg1_sb = cp.tile([GH, 1], F32)
        nc.gpsimd.dma_start(bg1_sb[:], bg1_d[:])
        wg2_sb = cp.tile([GH + 1, E], F32)
        nc.gpsimd.dma_start(wg2_sb[:], wg2_d[:])
        bexp_sb = cp.tile([E, 128], F32)
        nc.gpsimd.dma_start(bexp_sb[:], bexp_d[:])
        wpsS_sb = cp.tile([64, E, 3, C], F16)
        nc.gpsimd.dma_start(wpsS_sb[:], wpsS_d[:])
        wpsP_sb = cp.tile([128, E, 3, C], F16)

        def load_wpsP():
            nc.sync.dma_start(wpsP_sb[:], wpsP_d[:])

        pooled0, bots0 = _emit_sample_loads(nc, pools, 0, XX0, xs_ap, mid_sp=load_wpsP)
        consts = (wg1x2_sb, bg1_sb, wg2_sb, bexp_sb, wpsP_sb, wpsS_sb, ones)

        g0 = _emit_sample_gate(nc, pools, 0, pooled0, consts, h_exts[0])
        add_dep_helper(bots0[0].ins, g0[3].ins, sync=False,
                       reason="s0 late bottom DMAs after s0 softmax exp")

        _emit_conv_blocks(nc, pools, 0, XX0, *g0[:3], out_ap, range(0, 2))
        pooled1, bots1 = _emit_sample_loads(nc, pools, 1, XX1, xs_ap)
        _emit_conv_blocks(nc, pools, 0, XX0, *g0[:3], out_ap, range(2, 5))
        g1 = _emit_sample_gate(nc, pools, 1, pooled1, consts, h_exts[1])
        _emit_conv_blocks(nc, pools, 0, XX0, *g0[:3], out_ap, range(5, NB))
        _emit_conv_blocks(nc, pools, 1, XX1, *g1[:3], out_ap, range(0, NB))

    nc.compile()
    _cache["nc"] = nc
    return nc


def host_prep(x, wg1, bg1, wg2, bg2, w_exp, b_exp):
    """Host-side layout prep + per-core sharding. Returns in_maps list."""
    x = np.asarray(x, dtype=np.float32).astype(np.float16)
    xpad = np.zeros((B, C, HP, WP), dtype=np.float16)
    xpad[:, :, 1:129, 1:129] = x
    xpad = xpad.reshape(B, C, FLAT)
    wg1 = np.asarray(wg1, dtype=np.float32)
    bg1 = np.asarray(bg1, dtype=np.float32).reshape(GH, 1)
    wg2 = np.asarray(wg2, dtype=np.float32)
    bg2 = np.asarray(bg2, dtype=np.float32).reshape(1, E)
    w_exp = np.asarray(w_exp, dtype=np.float32)
    b_exp = np.asarray(b_exp, dtype=np.float32)

    # w_exp [E, O, I, KH, KW] -> wt [I, E, KH, KW, O]
    wt = np.transpose(w_exp, (2, 0, 3, 4, 1)).copy()
    # residual shortcut: out += x == each expert's center tap += I
    # (routing probs sum to 1 up to the reference's 1e-8 epsilon)
    wt[:, :, 1, 1, :] += np.eye(C, dtype=np.float32)[:, None, :]
    # paired taps [128, E, 3, O]: top partitions = dx=-1, bottom = dx=+1
    wpsP = np.concatenate([wt[:, :, :, 0, :], wt[:, :, :, 2, :]], axis=0)
    # single dx=0 taps [64, E, 3, O]
    wpsS = wt[:, :, :, 1, :]

    shared = {
        "wpsP": np.ascontiguousarray(wpsP.astype(np.float16)),
        "wpsS": np.ascontiguousarray(wpsS.astype(np.float16)),
        "wg1": np.ascontiguousarray(np.concatenate([wg1, wg1], axis=0) / (H * W)),
        "bg1": np.ascontiguousarray(bg1),
        "wg2": np.ascontiguousarray(np.concatenate([wg2, bg2], axis=0)),
        "b_exp": np.ascontiguousarray(np.concatenate([b_exp, b_exp], axis=1)),
    }
    return [
        {"xs": np.ascontiguousarray(xpad[SPB * k : SPB * (k + 1)]), **shared}
        for k in range(NCORES)
    ]


def kernel(x, wg1, bg1, wg2, bg2, w_exp, b_exp):
    nc = build_program()
    in_maps = host_prep(x, wg1, bg1, wg2, bg2, w_exp, b_exp)
    res = run_bass_kernel_spmd(nc, in_maps, list(range(NCORES)))
    return np.concatenate(
        [res.results[k]["out"].astype(np.float32) for k in range(NCORES)], axis=0
    )
